# revision 20
# baseline (speedup 1.0000x reference)
"""
Trainium2 Bass kernel for nn_DisjointDecoderAE.

  encoder (shared MLP):  x[B,U] -> relu x3 -> z3[B,H]   (enc L4 fused into dec L1)
  decoder (U disjoint MLPs): z3 -> relu(64) -> relu(64) -> relu(64) -> scalar

Sharding: encoder replicated on every core (tiny); decoder expert-parallel over
the unit axis (64 units/core x 8 cores).  Activations feature-on-partition /
batch-on-free.  The PSUM->SBUF drain (bias+relu) on VectorE+ScalarE is the hard
roofline (~123us combined); schedule keeps both engines saturated:
  - encoder L4 composed into decoder L1 weights (no relu between them)
  - encoder runs in 512-col chunks so decoder drains start ~8us in
  - decoder L1: K=64 fused weights, unit-pair per [64,128] lhsT, 2-way strips
  - L2/L3: 4-unit subgroups packed in PE quadrants (as before)
  - L4: 2 groups (16 units) accumulate-packed into one PSUM tile via a
    zero-weight clearing matmul + M=4 matmuls -> 8 copy drains total, and
    contiguous [4,2048] output DMA slices issued as each set completes.
"""

import os
import sys

sys.path.insert(0, "/opt/trn_rl_repo")

import numpy as np
import ml_dtypes

import concourse.bass as bass
import concourse.mybir as mybir
import concourse.tile as tile
from concourse import bacc
from concourse.bass_utils import run_bass_kernel_spmd

B, U, L, H = 2048, 512, 32, 64
NCORES = 8
UC = U // NCORES          # 64 units per core
NG = UC // 8              # 8 groups of 8 units
CH = 512                  # encoder chunk / MM free dim
CP = 1024                 # drain granularity (2 psum banks)
NCP = B // CP             # 2 chunk-pairs
KT = U // 128             # 4 k-tiles for encoder layer 1

BF16 = mybir.dt.bfloat16
FP32 = mybir.dt.float32
NPBF = ml_dtypes.bfloat16

# L2/L3 per-subgroup placement (j = unit index within subgroup of 4).
IH2 = (0, 1, 0, 1)
OH2 = (0, 1, 1, 0)
TS2 = (0, 0, 1, 1)
BK2 = (0, 0, 1, 1)
IH3 = (0, 1, 1, 0)
OH3 = (0, 1, 0, 1)
TS3 = (0, 0, 1, 1)
BK3 = (0, 0, 1, 1)

LAST_EXEC_NS = None
LAST_RESULTS = None
_PROG = None


NCH = B // CH             # 4 encoder chunks


def _pack_shared(x, We1, be1, We2, be2, We3, be3):
    xT = np.ascontiguousarray(x.T).astype(NPBF)              # [U, B]
    # chunk-major: xt[c, p, t, j] = xT[128t+p, 512c+j] -> one DMA per chunk
    xt = np.ascontiguousarray(
        xT.reshape(KT, 128, NCH, CH).transpose(2, 1, 0, 3))
    wenc = np.zeros((128, 448), np.float32)
    wenc[:, 0:KT * H] = We1.reshape(KT, 128, H).transpose(1, 0, 2).reshape(
        128, KT * H)
    wenc[0:H, 256:320] = We2
    wenc[0:H, 320:448] = np.tile(We3, (1, 2))                # z3 replicated x2
    benc = np.zeros((128, 3), np.float32)
    benc[0:H, 0] = be1
    benc[0:H, 1] = be2
    benc[:, 2] = np.tile(be3, 2)          # z3r is replicated x2 on partitions
    return dict(xt=xt, wenc=wenc.astype(NPBF), benc=benc)


def _pack_core(c, We4, be4, Wd1, bd1, Wd2, bd2, Wd3, bd3, Wd4):
    u0 = c * UC
    # fuse encoder L4 (no relu after it) into decoder L1:
    #   h1 = relu(Wf[u]^T z3 + bf[u]),  Wf[u] = We4 @ Wd1[u],
    #   bf[u] = be4 @ Wd1[u] + bd1[u]
    w1 = np.einsum('hl,ulk->uhk', We4.astype(np.float64),
                   Wd1[u0:u0 + UC].astype(np.float64)).astype(np.float32)
    b1 = (be4.astype(np.float64) @ Wd1[u0:u0 + UC].astype(np.float64)
          ).astype(np.float32) + bd1[u0:u0 + UC]
    w2 = Wd2[u0:u0 + UC]
    b2 = bd2[u0:u0 + UC]
    w3 = Wd3[u0:u0 + UC]
    b3 = bd3[u0:u0 + UC]
    w4 = Wd4[u0:u0 + UC]

    # L1: pair p -> lhsT [64,128] at row half p%2, col block p//2.
    wd1f = np.zeros((128, (UC // 4) * 128), np.float32)
    bd1p = np.zeros((128, UC // 2), np.float32)
    for p in range(UC // 2):
        hrow = 64 * (p % 2)
        blk = p // 2
        wd1f[hrow:hrow + 64, blk * 128:blk * 128 + 64] = w1[2 * p]
        wd1f[hrow:hrow + 64, blk * 128 + 64:blk * 128 + 128] = w1[2 * p + 1]
        bd1p[0:64, p] = b1[2 * p]
        bd1p[64:128, p] = b1[2 * p + 1]

    NS = UC // 4  # 16 subgroups
    wd2p = np.zeros((128, NS * 2 * H), np.float32)
    wd3p = np.zeros((128, NS * 2 * H), np.float32)
    bd2p = np.zeros((128, NS * 2), np.float32)
    bd3p = np.zeros((128, NS * 2), np.float32)
    for s in range(NS):
        for j in range(4):
            u = 4 * s + j
            blk = 2 * s + (j >> 1)
            wd2p[64 * IH2[j]:64 * IH2[j] + 64, blk * H:(blk + 1) * H] = w2[u]
            wd3p[64 * IH3[j]:64 * IH3[j] + 64, blk * H:(blk + 1) * H] = w3[u]
        # T2 banks: A = {4s lo, 4s+1 hi}; B = {4s+3 lo, 4s+2 hi}
        bd2p[0:64, 2 * s] = b2[4 * s]
        bd2p[64:128, 2 * s] = b2[4 * s + 1]
        bd2p[0:64, 2 * s + 1] = b2[4 * s + 3]
        bd2p[64:128, 2 * s + 1] = b2[4 * s + 2]
        bd3p[0:64, 2 * s] = b3[4 * s]
        bd3p[64:128, 2 * s] = b3[4 * s + 1]
        bd3p[0:64, 2 * s + 1] = b3[4 * s + 2]
        bd3p[64:128, 2 * s + 1] = b3[4 * s + 3]

    # L4: per 2-group set G (16 units, 8 local pairs p): pair -> col-group
    # p//2, within-block cols {2(p%2), 2(p%2)+1}; M=4 accumulate-packed.
    # psum row 32*(p//2) + 2*(p%2) + k  ==  unit 16G + 4*(p//2) + 2*(p%2)+k.
    wd4p = np.zeros((128, UC // 2 * 4), np.float32)
    for q in range(UC // 2):          # global pair index
        G = q // 8
        p = q % 8
        base = q * 4
        col0 = 2 * (p % 2)
        wd4p[0:64, base + col0] = w4[2 * q]
        wd4p[64:128, base + col0 + 1] = w4[2 * q + 1]

    bdec = np.concatenate([bd1p, bd2p, bd3p], axis=1)        # [128, 96]
    return dict(wd1=wd1f.astype(NPBF), wd2=wd2p.astype(NPBF),
                wd3=wd3p.astype(NPBF), wd4=wd4p.astype(NPBF), bdec=bdec)


class _Drain:
    """Weighted VectorE/ScalarE alternation for PSUM->SBUF drains,
    using HW-measured per-op costs."""

    def __init__(self, nc):
        self.nc = nc
        self.t_dve = 0.0
        self.t_act = 0.0

    def __call__(self, out, psum, bias=None, relu=False):
        fd = 1
        for step, cnt in psum.ap[1:]:
            fd *= cnt
        dve_ns = (120.0 + fd) / 0.96 + 88.0
        act_ns = (172.0 + fd) / 1.2 + 117.0
        nc = self.nc
        if self.t_dve + dve_ns <= self.t_act + act_ns:
            self.t_dve += dve_ns
            if relu:
                nc.vector.tensor_scalar(out, psum, bias, 0.0,
                                        op0=mybir.AluOpType.add,
                                        op1=mybir.AluOpType.max)
            elif bias is not None:
                nc.vector.tensor_scalar(out, psum, bias, None,
                                        op0=mybir.AluOpType.add)
            else:
                nc.vector.tensor_copy(out, psum)
        else:
            self.t_act += act_ns
            if relu:
                nc.scalar.activation(out, psum, mybir.ActivationFunctionType.Relu,
                                     bias=bias)
            elif bias is not None:
                nc.scalar.activation(out, psum,
                                     mybir.ActivationFunctionType.Identity,
                                     bias=bias)
            else:
                nc.scalar.copy(out, psum)


def _build_program():
    nc = bacc.Bacc("TRN2", target_bir_lowering=False, debug=False)

    def din(name, shape, dtype):
        return nc.dram_tensor(name, list(shape), dtype, kind="ExternalInput").ap()

    xt_d = din("xt", (NCH, 128, KT, CH), BF16)
    wenc_d = din("wenc", (128, 448), BF16)
    benc_d = din("benc", (128, 3), FP32)
    wd1_d = din("wd1", (128, (UC // 4) * 128), BF16)
    wd2_d = din("wd2", (128, UC // 2 * H), BF16)
    wd3_d = din("wd3", (128, UC // 2 * H), BF16)
    wd4_d = din("wd4", (128, UC // 2 * 4), BF16)
    bdec_d = din("bdec", (128, 96), FP32)
    out_d = nc.dram_tensor("out", [UC, B], BF16, kind="ExternalOutput").ap()

    RELU = True

    with tile.TileContext(nc) as tc:
        with (
            tc.tile_pool(name="const", bufs=1) as const,
            tc.tile_pool(name="h1p", bufs=2) as h1p,
            tc.tile_pool(name="h2p", bufs=2) as h2p,
            tc.tile_pool(name="h3p", bufs=3) as h3p,
            tc.tile_pool(name="stg", bufs=2) as stgp,
            tc.tile_pool(name="ps", bufs=4, space="PSUM") as psp,
        ):
            drain = _Drain(nc)

            def load(dst_shape, dtype, src, tag, eng=None):
                t = const.tile(list(dst_shape), dtype, tag=tag, name=tag)
                (eng or nc.sync).dma_start(out=t[:], in_=src)
                return t

            # PE warm-up on memset data (~3.4us to bring HAM to K=8/8).  wu
            # also serves as the zero lhsT for the L4 bank-clearing matmuls.
            wu = const.tile([128, 512], BF16, tag="wu", name="wu")
            nc.gpsimd.memset(wu[:], 0.0)
            wu_ps = psp.tile([128, CP], FP32, tag="ps", name="wu_ps")
            for i in range(18):
                nc.tensor.matmul(wu_ps[:, (i % 4) * 256:(i % 4) * 256 + 256],
                                 wu[:, 0:128], wu[:, 0:256])

            # x chunk 0 first on the sync HWDGE ring; small weights go on the
            # scalar HWDGE ring so issues overlap instead of serializing.
            xb = const.tile([128, KT * B], BF16, tag="xb", name="xb")
            xbr = xb[:].rearrange("p (t c) -> p t c", t=KT)
            # chunk 0 in two k-tile-pair halves so e1's first matmuls start
            # as soon as the first 256KB lands
            nc.sync.dma_start(out=xbr[:, 0:2, 0:CH], in_=xt_d[0][:, 0:2, :])
            nc.sync.dma_start(out=xbr[:, 2:4, 0:CH], in_=xt_d[0][:, 2:4, :])
            wenc = const.tile([128, 448], BF16, tag="wenc", name="wenc")
            nc.scalar.dma_start(out=wenc[:], in_=wenc_d[:])
            benc = const.tile([128, 3], FP32, tag="benc", name="benc")
            nc.scalar.dma_start(out=benc[:], in_=benc_d[:])
            for c in range(1, NCH):
                nc.sync.dma_start(out=xbr[:, :, c * CH:c * CH + CH],
                                  in_=xt_d[c])
            wd1 = load((128, (UC // 4) * 128), BF16, wd1_d[:], "wd1")
            bdec = load((128, 96), FP32, bdec_d[:], "bdec")

            we2 = wenc[0:H, 256:320]
            we3r = wenc[0:H, 320:448]
            be1 = benc[0:H, 0:1]
            be2 = benc[0:H, 1:2]
            be3 = benc[:, 2:3]

            def xsl(t, c0):
                return xb[:, t * B + c0:t * B + c0 + CH]

            wd2 = load((128, UC // 2 * H), BF16, wd2_d[:], "wd2")
            wd3 = load((128, UC // 2 * H), BF16, wd3_d[:], "wd3")
            wd4 = load((128, UC // 2 * 4), BF16, wd4_d[:], "wd4")
            bd1 = bdec[:, 0:32]
            bd2 = bdec[:, 32:64]
            bd3 = bdec[:, 64:96]

            z1 = const.tile([H, B], BF16, tag="z1", name="z1")
            z2 = const.tile([H, B], BF16, tag="z2", name="z2")
            z3r = const.tile([128, B], BF16, tag="z3r", name="z3r")

            # ---------------- encoder (replicated), 512-col chunks ----------
            def enc_half(half):
                # two chunks share each [128,CP] psum tile (cols cc*CH)
                p1 = psp.tile([128, CP], FP32, tag="ps", name="pe1")
                for cc in range(2):
                    c0 = (2 * half + cc) * CH
                    for t in range(KT):
                        nc.tensor.matmul(p1[0:H, cc * CH:cc * CH + CH],
                                         wenc[:, t * H:(t + 1) * H],
                                         xsl(t, c0),
                                         start=(t == 0), stop=(t == KT - 1))
                    drain(z1[:, c0:c0 + CH],
                          p1[0:H, cc * CH:cc * CH + CH], be1, RELU)
                p2 = psp.tile([128, CP], FP32, tag="ps", name="pe2")
                for cc in range(2):
                    c0 = (2 * half + cc) * CH
                    nc.tensor.matmul(p2[0:H, cc * CH:cc * CH + CH],
                                     we2, z1[:, c0:c0 + CH])
                    drain(z2[:, c0:c0 + CH],
                          p2[0:H, cc * CH:cc * CH + CH], be2, RELU)
                p3 = psp.tile([128, CP], FP32, tag="ps", name="pe3")
                for cc in range(2):
                    c0 = (2 * half + cc) * CH
                    nc.tensor.matmul(p3[:, cc * CH:cc * CH + CH],
                                     we3r, z2[:, c0:c0 + CH])
                    drain(z3r[:, c0:c0 + CH],
                          p3[:, cc * CH:cc * CH + CH], be3, RELU)

            # ---------------- decoder ----------------
            def l1_phase(g, d, cp, T1cp):
                # pairs 2d, 2d+1 (strips 0/1), both cc -> 2 psum tiles
                ps = [psp.tile([128, CP], FP32, tag="ps", name="pl1")
                      for _ in range(2)]
                for k in range(2):
                    p = 4 * g + 2 * d + k
                    hrow = 64 * (p % 2)
                    blk = p // 2
                    for cc in range(2):
                        c0 = cc * CH
                        nc.tensor.matmul(
                            ps[k][:, c0:c0 + CH],
                            wd1[hrow:hrow + 64, blk * 128:blk * 128 + 128],
                            z3r[hrow:hrow + 64,
                                cp * CP + c0:cp * CP + c0 + CH],
                            tile_position=(hrow, 0))
                for k in range(2):
                    p = 4 * g + 2 * d + k
                    drain(T1cp[2 * d + k][:, :], ps[k][:, :],
                          bd1[:, p:p + 1], RELU)

            def l2_phase(g, sloc, T1cp, T2cp, cp):
                s = 2 * g + sloc
                pa = psp.tile([128, CP], FP32, tag="ps", name="pa")
                pb = psp.tile([128, CP], FP32, tag="ps", name="pb")
                pp = (pa, pb)
                for cc in range(2):
                    c0 = cc * CH
                    for j in range(4):
                        blk = 2 * s + (j >> 1)
                        nc.tensor.matmul(
                            pp[BK2[j]][64 * OH2[j]:64 * OH2[j] + 64,
                                       c0:c0 + CH],
                            wd2[64 * IH2[j]:64 * IH2[j] + 64,
                                blk * H:(blk + 1) * H],
                            T1cp[2 * sloc + TS2[j]][
                                64 * IH2[j]:64 * IH2[j] + 64, c0:c0 + CH],
                            tile_position=(64 * IH2[j], 64 * OH2[j]))
                drain(T2cp[0][:, :], pa[:, :], bd2[:, 2 * s:2 * s + 1], RELU)
                drain(T2cp[1][:, :], pb[:, :],
                      bd2[:, 2 * s + 1:2 * s + 2], RELU)

            def l3_phase(g, sloc, T2cp, T3cp, cp):
                s = 2 * g + sloc
                pa = psp.tile([128, CP], FP32, tag="ps", name="pa3")
                pb = psp.tile([128, CP], FP32, tag="ps", name="pb3")
                pp = (pa, pb)
                for cc in range(2):
                    c0 = cc * CH
                    for j in range(4):
                        blk = 2 * s + (j >> 1)
                        nc.tensor.matmul(
                            pp[BK3[j]][64 * OH3[j]:64 * OH3[j] + 64,
                                       c0:c0 + CH],
                            wd3[64 * IH3[j]:64 * IH3[j] + 64,
                                blk * H:(blk + 1) * H],
                            T2cp[TS3[j]][64 * IH3[j]:64 * IH3[j] + 64,
                                         c0:c0 + CH],
                            tile_position=(64 * IH3[j], 64 * OH3[j]))
                drain(T3cp[0][:, :], pa[:, :], bd3[:, 2 * s:2 * s + 1], RELU)
                drain(T3cp[1][:, :], pb[:, :],
                      bd3[:, 2 * s + 1:2 * s + 2], RELU)

            # ---- L4 staged helpers (alloc+clear / accumulate mm / drain+DMA)
            l4ps = {}

            def l4_alloc(G, cp):
                p4 = psp.tile([128, CP], FP32, tag="ps", name="pl4")
                for cc in range(2):
                    c0 = cc * CH
                    nc.tensor.matmul(p4[:, c0:c0 + CH], wu[:, 0:128],
                                     wu[:, 0:CH], start=True, stop=False,
                                     skip_group_check=True)
                l4ps[(G, cp)] = p4

            def l4_mm(G, cp, T3l, prange, last_of_tile):
                # Pair p's lhsT [128,4] has data only in cols 2*(p%2)+{0,1};
                # the two pairs of col-group p//2 merge by PSUM accumulation
                # onto the zero-cleared bank.
                p4 = l4ps[(G, cp)]
                for cc in range(2):
                    c0 = cc * CH
                    for p in prange:
                        q = 8 * G + p            # global pair
                        cs = p // 2
                        lastmm = (cc == 1 and p == prange[-1]
                                  and last_of_tile)
                        nc.tensor.matmul(
                            p4[32 * cs:32 * cs + 4, c0:c0 + CH],
                            wd4[:, 4 * q:4 * q + 4],
                            T3l[p][:, c0:c0 + CH],
                            start=False, stop=lastmm,
                            skip_group_check=True,
                            tile_position=(0, 32 * cs))

            def l4_drain(G, cp, stg, last):
                p4 = l4ps.pop((G, cp))
                drain(stg[:, cp * CP:cp * CP + CP], p4[:, :], None, False)
                if last:
                    # tail: split each cp's strips across both HWDGE rings
                    for cs in range(4):
                        eng = nc.sync if cs % 2 == 0 else nc.scalar
                        eng.dma_start(
                            out=out_d[16 * G + 4 * cs:16 * G + 4 * cs + 4,
                                      cp * CP:cp * CP + CP],
                            in_=stg[32 * cs:32 * cs + 4,
                                    cp * CP:cp * CP + CP])
                elif cp == 1:
                    for cs in range(4):
                        nc.sync.dma_start(
                            out=out_d[16 * G + 4 * cs:16 * G + 4 * cs + 4,
                                      0:B],
                            in_=stg[32 * cs:32 * cs + 4, 0:2 * CP])

            def l4_block(G, cp, T3a, T3b, stg, last):
                l4_alloc(G, cp)
                l4_mm(G, cp, T3a[0][cp] + T3a[1][cp] +
                      T3b[0][cp] + T3b[1][cp], list(range(8)), True)
                l4_drain(G, cp, stg, last)

            def alloc_T1():
                return [[h1p.tile([128, CP], BF16, tag=f"t1_{k}_{cp}",
                                  name=f"t1_{k}_{cp}") for k in range(4)]
                        for cp in range(NCP)]

            enc_half(0)

            # group 0's L1 runs before the main loop (software pipeline);
            # encoder half 1 interleaves so its x-DMA-gated drains don't
            # block the ready L1 cp0 drains in the engine FIFOs.
            T1 = alloc_T1()
            for d in range(2):
                l1_phase(0, d, 0, T1[0])
            enc_half(1)
            for d in range(2):
                l1_phase(0, d, 1, T1[1])

            T3prev = None
            for g in range(NG):
                lastg = (g == NG - 1)
                T2 = [[[h2p.tile([128, CP], BF16, tag=f"t2_{sl_}_{k}_{cp}",
                                 name=f"t2_{sl_}_{k}_{cp}") for k in range(2)]
                       for cp in range(NCP)] for sl_ in range(2)]
                T3 = [[[h3p.tile([128, CP], BF16, tag=f"t3_{sl_}_{k}_{cp}",
                                 name=f"t3_{sl_}_{k}_{cp}") for k in range(2)]
                       for cp in range(NCP)] for sl_ in range(2)]
                if lastg:
                    stg_l = stgp.tile([128, 2 * CP], BF16, tag="stg",
                                      name="stg_last")
                    GL = g // 2

                if g == 0:
                    # cp0-first: the cp1 encoder/L1 chain completes late, so
                    # front-load all cp0 work to keep the drain engines fed
                    for cp in range(NCP):
                        for sloc in range(2):
                            l2_phase(g, sloc, T1[cp], T2[sloc][cp], cp)
                            l3_phase(g, sloc, T2[sloc][cp], T3[sloc][cp], cp)
                phase_iter = [] if g == 0 else [0, 1]
                for sloc in phase_iter:
                    for cp in range(NCP):
                        l2_phase(g, sloc, T1[cp], T2[sloc][cp], cp)
                        if lastg and sloc == 1 and cp == 1:
                            # last L4 set, cp0: pairs 0-3 come from group
                            # NG-2 whose T3 is long done.
                            l4_alloc(GL, 0)
                            l4_mm(GL, 0, T3prev[0][0] + T3prev[1][0] +
                                  [None] * 4, [0, 1, 2, 3], False)
                    for cp in range(NCP):
                        l3_phase(g, sloc, T2[sloc][cp], T3[sloc][cp], cp)
                        if lastg and sloc == 1 and cp == 0:
                            l4_mm(GL, 0, [None] * 4 +
                                  T3[0][0] + T3[1][0], [4, 5, 6, 7], True)
                            l4_drain(GL, 0, stg_l, True)

                # ---- end of group: next group's L1 interleaved with L4 ----
                if lastg:
                    l4_alloc(GL, 1)
                    l4_mm(GL, 1, T3prev[0][1] + T3prev[1][1] +
                          T3[0][1] + T3[1][1], list(range(8)), True)
                    l4_drain(GL, 1, stg_l, True)
                elif g % 2 == 1:
                    G = g // 2
                    stg = stgp.tile([128, 2 * CP], BF16, tag="stg",
                                    name=f"stg{G}")
                    T1n = alloc_T1()
                    l1_phase(g + 1, 0, 0, T1n[0])
                    l1_phase(g + 1, 1, 0, T1n[0])
                    l4_block(G, 0, T3prev, T3, stg, False)
                    l1_phase(g + 1, 0, 1, T1n[1])
                    l1_phase(g + 1, 1, 1, T1n[1])
                    l4_block(G, 1, T3prev, T3, stg, False)
                    T1 = T1n
                else:
                    T1n = alloc_T1()
                    for cp in range(NCP):
                        for d in range(2):
                            l1_phase(g + 1, d, cp, T1n[cp])
                    T1 = T1n
                T3prev = T3

    nc.compile()
    return nc


def _get_program():
    global _PROG
    if _PROG is None:
        _PROG = _build_program()
    return _PROG


def kernel(x, We1, be1, We2, be2, We3, be3, We4, be4,
           Wd1, bd1, Wd2, bd2, Wd3, bd3, Wd4, bd4):
    global LAST_EXEC_NS, LAST_RESULTS
    shared = _pack_shared(np.asarray(x, np.float32),
                          np.asarray(We1, np.float32), np.asarray(be1, np.float32),
                          np.asarray(We2, np.float32), np.asarray(be2, np.float32),
                          np.asarray(We3, np.float32), np.asarray(be3, np.float32))
    in_maps = []
    for c in range(NCORES):
        m = dict(shared)
        m.update(_pack_core(c, np.asarray(We4, np.float32), np.asarray(be4, np.float32),
                            np.asarray(Wd1, np.float32), np.asarray(bd1, np.float32),
                            np.asarray(Wd2, np.float32), np.asarray(bd2, np.float32),
                            np.asarray(Wd3, np.float32), np.asarray(bd3, np.float32),
                            np.asarray(Wd4, np.float32)))
        in_maps.append(m)

    nc = _get_program()
    trace = bool(int(os.environ.get("BASSK_TRACE", "0")))
    kwargs = {}
    if trace:
        kwargs["tmpdir"] = os.environ.get("BASSK_TMPDIR") or None
    res = run_bass_kernel_spmd(nc, in_maps, core_ids=list(range(NCORES)),
                               trace=trace, **kwargs)
    LAST_EXEC_NS = res.exec_time_ns
    LAST_RESULTS = res

    outT = np.concatenate([res.results[c]["out"] for c in range(NCORES)], axis=0)
    out = outT.T.astype(np.float32) + np.asarray(bd4, np.float32)[None, :]
    return out



# revision 21
# speedup vs baseline: 1.0136x; 1.0136x over previous
"""
Trainium2 Bass kernel for nn_DisjointDecoderAE.

  encoder (shared MLP):  x[B,U] -> relu x3 -> z3[B,H]   (enc L4 fused into dec L1)
  decoder (U disjoint MLPs): z3 -> relu(64) -> relu(64) -> relu(64) -> scalar

Sharding: encoder replicated on every core (tiny); decoder expert-parallel over
the unit axis (64 units/core x 8 cores).  Activations feature-on-partition /
batch-on-free.  The PSUM->SBUF drain (bias+relu) on VectorE+ScalarE is the hard
roofline (~123us combined); schedule keeps both engines saturated:
  - encoder L4 composed into decoder L1 weights (no relu between them)
  - encoder runs in 512-col chunks so decoder drains start ~8us in
  - decoder L1: K=64 fused weights, unit-pair per [64,128] lhsT, 2-way strips
  - L2/L3: 4-unit subgroups packed in PE quadrants (as before)
  - L4: 2 groups (16 units) accumulate-packed into one PSUM tile via a
    zero-weight clearing matmul + M=4 matmuls -> 8 copy drains total, and
    contiguous [4,2048] output DMA slices issued as each set completes.
"""

import os
import sys

sys.path.insert(0, "/opt/trn_rl_repo")

import numpy as np
import ml_dtypes

import concourse.bass as bass
import concourse.mybir as mybir
import concourse.tile as tile
from concourse import bacc
from concourse.bass_utils import run_bass_kernel_spmd

B, U, L, H = 2048, 512, 32, 64
NCORES = 8
UC = U // NCORES          # 64 units per core
NG = UC // 8              # 8 groups of 8 units
CH = 512                  # encoder chunk / MM free dim
CP = 1024                 # drain granularity (2 psum banks)
NCP = B // CP             # 2 chunk-pairs
KT = U // 128             # 4 k-tiles for encoder layer 1

BF16 = mybir.dt.bfloat16
FP32 = mybir.dt.float32
NPBF = ml_dtypes.bfloat16

# L2/L3 per-subgroup placement (j = unit index within subgroup of 4).
IH2 = (0, 1, 0, 1)
OH2 = (0, 1, 1, 0)
TS2 = (0, 0, 1, 1)
BK2 = (0, 0, 1, 1)
IH3 = (0, 1, 1, 0)
OH3 = (0, 1, 0, 1)
TS3 = (0, 0, 1, 1)
BK3 = (0, 0, 1, 1)

LAST_EXEC_NS = None
LAST_RESULTS = None
_PROG = None


NCH = B // CH             # 4 encoder chunks


def _pack_shared(x, We1, be1, We2, be2, We3, be3):
    xT = np.ascontiguousarray(x.T).astype(NPBF)              # [U, B]
    # chunk-major: xt[c, p, t, j] = xT[128t+p, 512c+j] -> one DMA per chunk
    xt = np.ascontiguousarray(
        xT.reshape(KT, 128, NCH, CH).transpose(2, 1, 0, 3))
    wenc = np.zeros((128, 448), np.float32)
    wenc[:, 0:KT * H] = We1.reshape(KT, 128, H).transpose(1, 0, 2).reshape(
        128, KT * H)
    wenc[0:H, 256:320] = We2
    wenc[0:H, 320:448] = np.tile(We3, (1, 2))                # z3 replicated x2
    benc = np.zeros((128, 3), np.float32)
    benc[0:H, 0] = be1
    benc[0:H, 1] = be2
    benc[:, 2] = np.tile(be3, 2)          # z3r is replicated x2 on partitions
    return dict(xt=xt, wenc=wenc.astype(NPBF), benc=benc)


def _pack_core(c, We4, be4, Wd1, bd1, Wd2, bd2, Wd3, bd3, Wd4):
    u0 = c * UC
    # fuse encoder L4 (no relu after it) into decoder L1:
    #   h1 = relu(Wf[u]^T z3 + bf[u]),  Wf[u] = We4 @ Wd1[u],
    #   bf[u] = be4 @ Wd1[u] + bd1[u]
    w1 = np.einsum('hl,ulk->uhk', We4.astype(np.float64),
                   Wd1[u0:u0 + UC].astype(np.float64)).astype(np.float32)
    b1 = (be4.astype(np.float64) @ Wd1[u0:u0 + UC].astype(np.float64)
          ).astype(np.float32) + bd1[u0:u0 + UC]
    w2 = Wd2[u0:u0 + UC]
    b2 = bd2[u0:u0 + UC]
    w3 = Wd3[u0:u0 + UC]
    b3 = bd3[u0:u0 + UC]
    w4 = Wd4[u0:u0 + UC]

    # L1: pair p -> lhsT [64,128] at row half p%2, col block p//2.
    wd1f = np.zeros((128, (UC // 4) * 128), np.float32)
    bd1p = np.zeros((128, UC // 2), np.float32)
    for p in range(UC // 2):
        hrow = 64 * (p % 2)
        blk = p // 2
        wd1f[hrow:hrow + 64, blk * 128:blk * 128 + 64] = w1[2 * p]
        wd1f[hrow:hrow + 64, blk * 128 + 64:blk * 128 + 128] = w1[2 * p + 1]
        bd1p[0:64, p] = b1[2 * p]
        bd1p[64:128, p] = b1[2 * p + 1]

    NS = UC // 4  # 16 subgroups
    wd2p = np.zeros((128, NS * 2 * H), np.float32)
    wd3p = np.zeros((128, NS * 2 * H), np.float32)
    bd2p = np.zeros((128, NS * 2), np.float32)
    bd3p = np.zeros((128, NS * 2), np.float32)
    for s in range(NS):
        for j in range(4):
            u = 4 * s + j
            blk = 2 * s + (j >> 1)
            wd2p[64 * IH2[j]:64 * IH2[j] + 64, blk * H:(blk + 1) * H] = w2[u]
            wd3p[64 * IH3[j]:64 * IH3[j] + 64, blk * H:(blk + 1) * H] = w3[u]
        # T2 banks: A = {4s lo, 4s+1 hi}; B = {4s+3 lo, 4s+2 hi}
        bd2p[0:64, 2 * s] = b2[4 * s]
        bd2p[64:128, 2 * s] = b2[4 * s + 1]
        bd2p[0:64, 2 * s + 1] = b2[4 * s + 3]
        bd2p[64:128, 2 * s + 1] = b2[4 * s + 2]
        bd3p[0:64, 2 * s] = b3[4 * s]
        bd3p[64:128, 2 * s] = b3[4 * s + 1]
        bd3p[0:64, 2 * s + 1] = b3[4 * s + 2]
        bd3p[64:128, 2 * s + 1] = b3[4 * s + 3]

    # L4: per 2-group set G (16 units, 8 local pairs p): pair -> col-group
    # p//2, within-block cols {2(p%2), 2(p%2)+1}; M=4 accumulate-packed.
    # psum row 32*(p//2) + 2*(p%2) + k  ==  unit 16G + 4*(p//2) + 2*(p%2)+k.
    wd4p = np.zeros((128, UC // 2 * 4), np.float32)
    for q in range(UC // 2):          # global pair index
        G = q // 8
        p = q % 8
        base = q * 4
        col0 = 2 * (p % 2)
        wd4p[0:64, base + col0] = w4[2 * q]
        wd4p[64:128, base + col0 + 1] = w4[2 * q + 1]

    bdec = np.concatenate([bd1p, bd2p, bd3p], axis=1)        # [128, 96]
    return dict(wd1=wd1f.astype(NPBF), wd2=wd2p.astype(NPBF),
                wd3=wd3p.astype(NPBF), wd4=wd4p.astype(NPBF), bdec=bdec)


class _Drain:
    """Weighted VectorE/ScalarE alternation for PSUM->SBUF drains,
    using HW-measured per-op costs."""

    def __init__(self, nc):
        self.nc = nc
        self.t_dve = 0.0
        self.t_act = 0.0

    def __call__(self, out, psum, bias=None, relu=False):
        fd = 1
        for step, cnt in psum.ap[1:]:
            fd *= cnt
        dve_ns = (120.0 + fd) / 0.96 + 88.0
        act_ns = (172.0 + fd) / 1.2 + 117.0
        nc = self.nc
        if self.t_dve + dve_ns <= self.t_act + act_ns:
            self.t_dve += dve_ns
            if relu:
                nc.vector.tensor_scalar(out, psum, bias, 0.0,
                                        op0=mybir.AluOpType.add,
                                        op1=mybir.AluOpType.max)
            elif bias is not None:
                nc.vector.tensor_scalar(out, psum, bias, None,
                                        op0=mybir.AluOpType.add)
            else:
                nc.vector.tensor_copy(out, psum)
        else:
            self.t_act += act_ns
            if relu:
                nc.scalar.activation(out, psum, mybir.ActivationFunctionType.Relu,
                                     bias=bias)
            elif bias is not None:
                nc.scalar.activation(out, psum,
                                     mybir.ActivationFunctionType.Identity,
                                     bias=bias)
            else:
                nc.scalar.copy(out, psum)


def _build_program():
    nc = bacc.Bacc("TRN2", target_bir_lowering=False, debug=False)

    def din(name, shape, dtype):
        return nc.dram_tensor(name, list(shape), dtype, kind="ExternalInput").ap()

    xt_d = din("xt", (NCH, 128, KT, CH), BF16)
    wenc_d = din("wenc", (128, 448), BF16)
    benc_d = din("benc", (128, 3), FP32)
    wd1_d = din("wd1", (128, (UC // 4) * 128), BF16)
    wd2_d = din("wd2", (128, UC // 2 * H), BF16)
    wd3_d = din("wd3", (128, UC // 2 * H), BF16)
    wd4_d = din("wd4", (128, UC // 2 * 4), BF16)
    bdec_d = din("bdec", (128, 96), FP32)
    out_d = nc.dram_tensor("out", [UC, B], BF16, kind="ExternalOutput").ap()

    RELU = True

    with tile.TileContext(nc) as tc:
        with (
            tc.tile_pool(name="const", bufs=1) as const,
            tc.tile_pool(name="h1p", bufs=2) as h1p,
            tc.tile_pool(name="h2p", bufs=2) as h2p,
            tc.tile_pool(name="h3p", bufs=3) as h3p,
            tc.tile_pool(name="stg", bufs=2) as stgp,
            tc.tile_pool(name="ps", bufs=4, space="PSUM") as psp,
        ):
            drain = _Drain(nc)

            def load(dst_shape, dtype, src, tag, eng=None):
                t = const.tile(list(dst_shape), dtype, tag=tag, name=tag)
                (eng or nc.sync).dma_start(out=t[:], in_=src)
                return t

            # PE warm-up on memset data (~3.4us to bring HAM to K=8/8).  wu
            # also serves as the zero lhsT for the L4 bank-clearing matmuls.
            wu = const.tile([128, 512], BF16, tag="wu", name="wu")
            nc.gpsimd.memset(wu[:], 0.0)
            wu_ps = psp.tile([128, CP], FP32, tag="ps", name="wu_ps")
            for i in range(18):
                nc.tensor.matmul(wu_ps[:, (i % 4) * 256:(i % 4) * 256 + 256],
                                 wu[:, 0:128], wu[:, 0:256])

            # x chunk 0 first on the sync HWDGE ring; small weights go on the
            # scalar HWDGE ring so issues overlap instead of serializing.
            xb = const.tile([128, KT * B], BF16, tag="xb", name="xb")
            xbr = xb[:].rearrange("p (t c) -> p t c", t=KT)
            # chunk 0 in two k-tile-pair halves so e1's first matmuls start
            # as soon as the first 256KB lands
            nc.sync.dma_start(out=xbr[:, 0:2, 0:CH], in_=xt_d[0][:, 0:2, :])
            nc.sync.dma_start(out=xbr[:, 2:4, 0:CH], in_=xt_d[0][:, 2:4, :])
            wenc = const.tile([128, 448], BF16, tag="wenc", name="wenc")
            nc.scalar.dma_start(out=wenc[:], in_=wenc_d[:])
            benc = const.tile([128, 3], FP32, tag="benc", name="benc")
            nc.scalar.dma_start(out=benc[:], in_=benc_d[:])
            nc.scalar.dma_start(out=xbr[:, :, CH:2 * CH], in_=xt_d[1])
            nc.sync.dma_start(out=xbr[:, :, 2 * CH:3 * CH], in_=xt_d[2])
            nc.scalar.dma_start(out=xbr[:, :, 3 * CH:4 * CH], in_=xt_d[3])
            wd1 = load((128, (UC // 4) * 128), BF16, wd1_d[:], "wd1")
            bdec = load((128, 96), FP32, bdec_d[:], "bdec")

            we2 = wenc[0:H, 256:320]
            we3r = wenc[0:H, 320:448]
            be1 = benc[0:H, 0:1]
            be2 = benc[0:H, 1:2]
            be3 = benc[:, 2:3]

            def xsl(t, c0):
                return xb[:, t * B + c0:t * B + c0 + CH]

            wd2 = load((128, UC // 2 * H), BF16, wd2_d[:], "wd2")
            wd3 = load((128, UC // 2 * H), BF16, wd3_d[:], "wd3")
            wd4 = load((128, UC // 2 * 4), BF16, wd4_d[:], "wd4")
            bd1 = bdec[:, 0:32]
            bd2 = bdec[:, 32:64]
            bd3 = bdec[:, 64:96]

            z1 = const.tile([H, B], BF16, tag="z1", name="z1")
            z2 = const.tile([H, B], BF16, tag="z2", name="z2")
            z3r = const.tile([128, B], BF16, tag="z3r", name="z3r")

            # ---------------- encoder (replicated), 512-col chunks ----------
            def enc_half(half):
                # two chunks share each [128,CP] psum tile (cols cc*CH)
                p1 = psp.tile([128, CP], FP32, tag="ps", name="pe1")
                for cc in range(2):
                    c0 = (2 * half + cc) * CH
                    for t in range(KT):
                        nc.tensor.matmul(p1[0:H, cc * CH:cc * CH + CH],
                                         wenc[:, t * H:(t + 1) * H],
                                         xsl(t, c0),
                                         start=(t == 0), stop=(t == KT - 1))
                    drain(z1[:, c0:c0 + CH],
                          p1[0:H, cc * CH:cc * CH + CH], be1, RELU)
                p2 = psp.tile([128, CP], FP32, tag="ps", name="pe2")
                for cc in range(2):
                    c0 = (2 * half + cc) * CH
                    nc.tensor.matmul(p2[0:H, cc * CH:cc * CH + CH],
                                     we2, z1[:, c0:c0 + CH])
                    drain(z2[:, c0:c0 + CH],
                          p2[0:H, cc * CH:cc * CH + CH], be2, RELU)
                p3 = psp.tile([128, CP], FP32, tag="ps", name="pe3")
                for cc in range(2):
                    c0 = (2 * half + cc) * CH
                    nc.tensor.matmul(p3[:, cc * CH:cc * CH + CH],
                                     we3r, z2[:, c0:c0 + CH])
                    drain(z3r[:, c0:c0 + CH],
                          p3[:, cc * CH:cc * CH + CH], be3, RELU)

            # ---------------- decoder ----------------
            def l1_phase(g, d, cp, T1cp):
                # pairs 2d, 2d+1 (strips 0/1), both cc -> 2 psum tiles
                ps = [psp.tile([128, CP], FP32, tag="ps", name="pl1")
                      for _ in range(2)]
                for k in range(2):
                    p = 4 * g + 2 * d + k
                    hrow = 64 * (p % 2)
                    blk = p // 2
                    for cc in range(2):
                        c0 = cc * CH
                        nc.tensor.matmul(
                            ps[k][:, c0:c0 + CH],
                            wd1[hrow:hrow + 64, blk * 128:blk * 128 + 128],
                            z3r[hrow:hrow + 64,
                                cp * CP + c0:cp * CP + c0 + CH],
                            tile_position=(hrow, 0))
                for k in range(2):
                    p = 4 * g + 2 * d + k
                    drain(T1cp[2 * d + k][:, :], ps[k][:, :],
                          bd1[:, p:p + 1], RELU)

            def l2_phase(g, sloc, T1cp, T2cp, cp):
                s = 2 * g + sloc
                pa = psp.tile([128, CP], FP32, tag="ps", name="pa")
                pb = psp.tile([128, CP], FP32, tag="ps", name="pb")
                pp = (pa, pb)
                for cc in range(2):
                    c0 = cc * CH
                    for j in range(4):
                        blk = 2 * s + (j >> 1)
                        nc.tensor.matmul(
                            pp[BK2[j]][64 * OH2[j]:64 * OH2[j] + 64,
                                       c0:c0 + CH],
                            wd2[64 * IH2[j]:64 * IH2[j] + 64,
                                blk * H:(blk + 1) * H],
                            T1cp[2 * sloc + TS2[j]][
                                64 * IH2[j]:64 * IH2[j] + 64, c0:c0 + CH],
                            tile_position=(64 * IH2[j], 64 * OH2[j]))
                drain(T2cp[0][:, :], pa[:, :], bd2[:, 2 * s:2 * s + 1], RELU)
                drain(T2cp[1][:, :], pb[:, :],
                      bd2[:, 2 * s + 1:2 * s + 2], RELU)

            def l3_phase(g, sloc, T2cp, T3cp, cp):
                s = 2 * g + sloc
                pa = psp.tile([128, CP], FP32, tag="ps", name="pa3")
                pb = psp.tile([128, CP], FP32, tag="ps", name="pb3")
                pp = (pa, pb)
                for cc in range(2):
                    c0 = cc * CH
                    for j in range(4):
                        blk = 2 * s + (j >> 1)
                        nc.tensor.matmul(
                            pp[BK3[j]][64 * OH3[j]:64 * OH3[j] + 64,
                                       c0:c0 + CH],
                            wd3[64 * IH3[j]:64 * IH3[j] + 64,
                                blk * H:(blk + 1) * H],
                            T2cp[TS3[j]][64 * IH3[j]:64 * IH3[j] + 64,
                                         c0:c0 + CH],
                            tile_position=(64 * IH3[j], 64 * OH3[j]))
                drain(T3cp[0][:, :], pa[:, :], bd3[:, 2 * s:2 * s + 1], RELU)
                drain(T3cp[1][:, :], pb[:, :],
                      bd3[:, 2 * s + 1:2 * s + 2], RELU)

            # ---- L4 staged helpers (alloc+clear / accumulate mm / drain+DMA)
            l4ps = {}

            def l4_alloc(G, cp):
                p4 = psp.tile([128, CP], FP32, tag="ps", name="pl4")
                for cc in range(2):
                    c0 = cc * CH
                    nc.tensor.matmul(p4[:, c0:c0 + CH], wu[:, 0:128],
                                     wu[:, 0:CH], start=True, stop=False,
                                     skip_group_check=True)
                l4ps[(G, cp)] = p4

            def l4_mm(G, cp, T3l, prange, last_of_tile):
                # Pair p's lhsT [128,4] has data only in cols 2*(p%2)+{0,1};
                # the two pairs of col-group p//2 merge by PSUM accumulation
                # onto the zero-cleared bank.
                p4 = l4ps[(G, cp)]
                for cc in range(2):
                    c0 = cc * CH
                    for p in prange:
                        q = 8 * G + p            # global pair
                        cs = p // 2
                        lastmm = (cc == 1 and p == prange[-1]
                                  and last_of_tile)
                        nc.tensor.matmul(
                            p4[32 * cs:32 * cs + 4, c0:c0 + CH],
                            wd4[:, 4 * q:4 * q + 4],
                            T3l[p][:, c0:c0 + CH],
                            start=False, stop=lastmm,
                            skip_group_check=True,
                            tile_position=(0, 32 * cs))

            def l4_drain(G, cp, stg, last):
                p4 = l4ps.pop((G, cp))
                drain(stg[:, cp * CP:cp * CP + CP], p4[:, :], None, False)
                if last:
                    # tail: split each cp's strips across both HWDGE rings
                    for cs in range(4):
                        eng = nc.sync if cs % 2 == 0 else nc.scalar
                        eng.dma_start(
                            out=out_d[16 * G + 4 * cs:16 * G + 4 * cs + 4,
                                      cp * CP:cp * CP + CP],
                            in_=stg[32 * cs:32 * cs + 4,
                                    cp * CP:cp * CP + CP])
                elif cp == 1:
                    for cs in range(4):
                        nc.sync.dma_start(
                            out=out_d[16 * G + 4 * cs:16 * G + 4 * cs + 4,
                                      0:B],
                            in_=stg[32 * cs:32 * cs + 4, 0:2 * CP])

            def l4_block(G, cp, T3a, T3b, stg, last):
                l4_alloc(G, cp)
                l4_mm(G, cp, T3a[0][cp] + T3a[1][cp] +
                      T3b[0][cp] + T3b[1][cp], list(range(8)), True)
                l4_drain(G, cp, stg, last)

            def alloc_T1():
                return [[h1p.tile([128, CP], BF16, tag=f"t1_{k}_{cp}",
                                  name=f"t1_{k}_{cp}") for k in range(4)]
                        for cp in range(NCP)]

            enc_half(0)

            # Software pipeline preamble: groups 0 and 1's L1-cp0 phases run
            # off encoder half 0 alone, giving the drain engines a deep queue
            # while the encoder-half-1 chain (gated on x chunks 2,3) runs.
            T1 = alloc_T1()
            T1b = alloc_T1()
            for d in range(2):
                l1_phase(0, d, 0, T1[0])
            for d in range(2):
                l1_phase(1, d, 0, T1b[0])
            enc_half(1)
            for d in range(2):
                l1_phase(0, d, 1, T1[1])
            for d in range(2):
                l1_phase(1, d, 1, T1b[1])

            T3prev = None
            for g in range(NG):
                lastg = (g == NG - 1)
                T2 = [[[h2p.tile([128, CP], BF16, tag=f"t2_{sl_}_{k}_{cp}",
                                 name=f"t2_{sl_}_{k}_{cp}") for k in range(2)]
                       for cp in range(NCP)] for sl_ in range(2)]
                T3 = [[[h3p.tile([128, CP], BF16, tag=f"t3_{sl_}_{k}_{cp}",
                                 name=f"t3_{sl_}_{k}_{cp}") for k in range(2)]
                       for cp in range(NCP)] for sl_ in range(2)]
                if lastg:
                    stg_l = stgp.tile([128, 2 * CP], BF16, tag="stg",
                                      name="stg_last")
                    GL = g // 2

                for sloc in range(2):
                    for cp in range(NCP):
                        l2_phase(g, sloc, T1[cp], T2[sloc][cp], cp)
                        if lastg and sloc == 1 and cp == 1:
                            # last L4 set, cp0: pairs 0-3 come from group
                            # NG-2 whose T3 is long done.
                            l4_alloc(GL, 0)
                            l4_mm(GL, 0, T3prev[0][0] + T3prev[1][0] +
                                  [None] * 4, [0, 1, 2, 3], False)
                    for cp in range(NCP):
                        l3_phase(g, sloc, T2[sloc][cp], T3[sloc][cp], cp)
                        if lastg and sloc == 1 and cp == 0:
                            l4_mm(GL, 0, [None] * 4 +
                                  T3[0][0] + T3[1][0], [4, 5, 6, 7], True)
                            l4_drain(GL, 0, stg_l, True)

                # ---- end of group: next group's L1 interleaved with L4 ----
                if g == 0:
                    T1 = T1b
                elif lastg:
                    l4_alloc(GL, 1)
                    l4_mm(GL, 1, T3prev[0][1] + T3prev[1][1] +
                          T3[0][1] + T3[1][1], list(range(8)), True)
                    l4_drain(GL, 1, stg_l, True)
                elif g % 2 == 1:
                    G = g // 2
                    stg = stgp.tile([128, 2 * CP], BF16, tag="stg",
                                    name=f"stg{G}")
                    T1n = alloc_T1()
                    l1_phase(g + 1, 0, 0, T1n[0])
                    l1_phase(g + 1, 1, 0, T1n[0])
                    l4_block(G, 0, T3prev, T3, stg, False)
                    l1_phase(g + 1, 0, 1, T1n[1])
                    l1_phase(g + 1, 1, 1, T1n[1])
                    l4_block(G, 1, T3prev, T3, stg, False)
                    T1 = T1n
                else:
                    T1n = alloc_T1()
                    for cp in range(NCP):
                        for d in range(2):
                            l1_phase(g + 1, d, cp, T1n[cp])
                    T1 = T1n
                T3prev = T3

    nc.compile()
    return nc


def _get_program():
    global _PROG
    if _PROG is None:
        _PROG = _build_program()
    return _PROG


def kernel(x, We1, be1, We2, be2, We3, be3, We4, be4,
           Wd1, bd1, Wd2, bd2, Wd3, bd3, Wd4, bd4):
    global LAST_EXEC_NS, LAST_RESULTS
    shared = _pack_shared(np.asarray(x, np.float32),
                          np.asarray(We1, np.float32), np.asarray(be1, np.float32),
                          np.asarray(We2, np.float32), np.asarray(be2, np.float32),
                          np.asarray(We3, np.float32), np.asarray(be3, np.float32))
    in_maps = []
    for c in range(NCORES):
        m = dict(shared)
        m.update(_pack_core(c, np.asarray(We4, np.float32), np.asarray(be4, np.float32),
                            np.asarray(Wd1, np.float32), np.asarray(bd1, np.float32),
                            np.asarray(Wd2, np.float32), np.asarray(bd2, np.float32),
                            np.asarray(Wd3, np.float32), np.asarray(bd3, np.float32),
                            np.asarray(Wd4, np.float32)))
        in_maps.append(m)

    nc = _get_program()
    trace = bool(int(os.environ.get("BASSK_TRACE", "0")))
    kwargs = {}
    if trace:
        kwargs["tmpdir"] = os.environ.get("BASSK_TMPDIR") or None
    res = run_bass_kernel_spmd(nc, in_maps, core_ids=list(range(NCORES)),
                               trace=trace, **kwargs)
    LAST_EXEC_NS = res.exec_time_ns
    LAST_RESULTS = res

    outT = np.concatenate([res.results[c]["out"] for c in range(NCORES)], axis=0)
    out = outT.T.astype(np.float32) + np.asarray(bd4, np.float32)[None, :]
    return out

